# revision 1
# baseline (speedup 1.0000x reference)
"""Trainium2 Bass kernel for the sketched-attention RS_SM op.

Reference semantics (per (b,h) pair):
    X  = concat([Q, K], axis=seq)                      # [4096, 64]
    XS = gather of 1024 landmark rows of X             # [m=4, d=256, 64]
    AS[n, d] = sum_m sign[m, d] * exp(X[n] . XS[m, d]) # [4096, 256]

Sharding: 16 (b,h) pairs over 8 cores = 2 pairs/core, no cross-core comms.

Device pipeline per (token-chunk t of 512, pair):
  MM1  : TensorE, lhsT = landmarksT [64, 128] (stationary), rhs = X^T [64, 512]
         -> PSUM [128 lmk, 512 n].  Two pairs row-tiled (K=64 each).
  exp  : ScalarE activation PSUM -> SBUF (bf16), grouped over 3 PSUM banks.
  MM2  : TensorE, lhsT = sign-delta W [128, 32], rhs = exp tile [128, 512]
         -> PSUM [32 d, 512 n], 4 col-tiled per output half -> [128, 512].
         This performs the signed reduction over m on the TensorE.
  copy : VectorE PSUM -> SBUF, DMA out [pair, 256, 4096] (d-major).

Landmark order is permuted (host-side) so chunk c holds (m, dl) for
d = 32c + dl: partition p = 32*m + dl.  W[32m+dl, 32c+dl] = sign[m, 32c+dl].
Host transposes the [256, 4096] device output to [4096, 256] at unshard.

All three device inputs (X^T | landmarks^T | W) are packed into one
[128, 5376] array so a single DMA (one semaphore lane) feeds the PE --
multiple DMA waits on one fused-LDW matmul overflow its sync-wait slots.
"""

import os
import sys
import types
from contextlib import ExitStack

import numpy as np

sys.path.insert(0, "/opt/trn_rl_repo")

# The axon client in this container lacks the NTFF profile hook module;
# provide a stub so bass_utils' trace path degrades gracefully.
try:
    import antenv.axon_hooks  # noqa: F401
except ImportError:
    _stub = types.ModuleType("antenv.axon_hooks")
    _stub.get_axon_ntff_profile_hook = lambda: None
    sys.modules["antenv.axon_hooks"] = _stub

import concourse.bacc as bacc
import concourse.bass as bass
import concourse.mybir as mybir
import concourse.tile as tile

B, H, N, P = 2, 8, 2048, 64
M, D = 4, 256
SEQ2 = 2 * N                      # 4096 tokens per pair
NCORES = 8
PAIRS = (B * H) // NCORES         # 2 pairs per core
L = M * D                         # 1024 landmarks per pair
TCH = 512                         # token chunk (matmul moving dim)
NT = SEQ2 // TCH                  # 8 token chunks
INW = SEQ2 + L + D                # packed input width: xt | lt | w
F32 = mybir.dt.float32
F32R = mybir.dt.float32r
BF16 = mybir.dt.bfloat16

# "bf16": cast inputs to bf16; "f32r": fp32 data through the PE fast path.
MM_MODE = os.environ.get("KERNEL_MM_MODE", "bf16")

_nc_cache = {}


def _build_nc():
    nc = bacc.Bacc(
        "TRN2", target_bir_lowering=False, debug=False, num_devices=NCORES,
    )
    in_dt = BF16 if MM_MODE == "bf16" else F32R

    inp = nc.dram_tensor("inp", [128, INW], in_dt, kind="ExternalInput")
    out = nc.dram_tensor("out", [PAIRS, D, SEQ2], F32, kind="ExternalOutput")

    with tile.TileContext(nc) as tc, ExitStack() as ctx:
        const_pool = ctx.enter_context(tc.tile_pool(name="const", bufs=1))
        inp_sb = const_pool.tile([128, INW], in_dt)
        nc.sync.dma_start(inp_sb[:], inp[:])
        xt_sb = inp_sb[:, 0:SEQ2]
        lt_sb = inp_sb[:, SEQ2:SEQ2 + L]
        w_sb = inp_sb[:, SEQ2 + L:INW]

        eps_pool = ctx.enter_context(tc.tile_pool(name="eps", bufs=2, space="PSUM"))
        asps_pool = ctx.enter_context(tc.tile_pool(name="asps", bufs=2, space="PSUM"))
        esb_pool = ctx.enter_context(tc.tile_pool(name="esb", bufs=2))
        assb_pool = ctx.enter_context(tc.tile_pool(name="assb", bufs=3))

        GROUPS = [(0, 1, 2), (3, 4, 5), (6, 7)]

        for t in range(NT):
            for pr in range(PAIRS):
                rows = slice(64 * pr, 64 * (pr + 1))
                rhs_x = xt_sb[rows, t * TCH:(t + 1) * TCH]
                # exp output for this (t, pair): free dim = 512*c + n
                e_sb = esb_pool.tile([128, 8 * TCH], in_dt, tag="esb")
                for grp in GROUPS:
                    e_ps = eps_pool.tile([128, TCH * 3], F32, tag="eps")
                    for gi, c in enumerate(grp):
                        nc.tensor.matmul(
                            e_ps[:, gi * TCH:(gi + 1) * TCH],
                            lhsT=lt_sb[rows, 128 * c:128 * (c + 1)],
                            rhs=rhs_x,
                            start=True, stop=True,
                            tile_position=(64 * pr, 0),
                        )
                    nc.scalar.activation(
                        e_sb[:, grp[0] * TCH:(grp[-1] + 1) * TCH],
                        e_ps[:, :TCH * len(grp)],
                        mybir.ActivationFunctionType.Exp,
                    )
                as_sb = assb_pool.tile([128, 2 * TCH], F32, tag="assb")
                for half in range(2):
                    as_ps = asps_pool.tile([128, TCH], F32, tag="asps")
                    for j in range(4):
                        c = 4 * half + j
                        nc.tensor.matmul(
                            as_ps[32 * j:32 * (j + 1), :],
                            lhsT=w_sb[:, 32 * c:32 * (c + 1)],
                            rhs=e_sb[:, c * TCH:(c + 1) * TCH],
                            start=True, stop=True,
                            tile_position=(0, 32 * j),
                        )
                    nc.vector.tensor_copy(
                        as_sb[:, half * TCH:(half + 1) * TCH], as_ps[:]
                    )
                hbm = out[pr].rearrange("(h p) n -> p h n", p=128)[
                    :, :, t * TCH:(t + 1) * TCH
                ]
                sb = as_sb[:].rearrange("p (h n) -> p h n", h=2)
                nc.sync.dma_start(hbm, sb)
    nc.compile()
    return nc


def _get_nc():
    key = MM_MODE
    if key not in _nc_cache:
        _nc_cache[key] = _build_nc()
    return _nc_cache[key]


_runner_cache = {}


def _get_runner():
    """Build (once) a jitted shard_map callable over the 8 cores, mirroring
    bass2jax.run_bass_via_pjrt but cached so repeat calls don't re-trace."""
    key = MM_MODE
    if key in _runner_cache:
        return _runner_cache[key]
    import jax
    from jax.sharding import Mesh, PartitionSpec
    try:
        from jax.experimental.shard_map import shard_map
    except ImportError:
        from jax.shard_map import shard_map  # newer jax
    from concourse import bass2jax as b2j

    b2j.install_neuronx_cc_hook()
    nc = _get_nc()

    partition_name = (
        nc.partition_id_tensor.name if nc.partition_id_tensor else None
    )
    in_names, out_names, out_avals, zero_shapes = [], [], [], []
    for alloc in nc.m.functions[0].allocations:
        if not isinstance(alloc, mybir.MemoryLocationSet):
            continue
        name = alloc.memorylocations[0].name
        if alloc.kind == "ExternalInput":
            if name != partition_name:
                in_names.append(name)
        elif alloc.kind == "ExternalOutput":
            out_names.append(name)
            shape = tuple(alloc.tensor_shape)
            dtype = mybir.dt.np(alloc.dtype)
            out_avals.append(jax.core.ShapedArray(shape, dtype))
            zero_shapes.append((shape, dtype))
    n_params = len(in_names)
    n_outs = len(out_avals)
    all_names = list(in_names) + list(out_names)
    if partition_name is not None:
        all_names.append(partition_name)
    donate = tuple(range(n_params, n_params + n_outs))

    def _body(*args):
        operands = list(args)
        if partition_name is not None:
            operands.append(b2j.partition_id_tensor())
        outs = b2j._bass_exec_p.bind(
            *operands,
            out_avals=tuple(out_avals),
            in_names=tuple(all_names),
            out_names=tuple(out_names),
            lowering_input_output_aliases=(),
            sim_require_finite=True,
            sim_require_nnan=True,
            nc=nc,
        )
        return tuple(outs)

    devices = jax.devices()[:NCORES]
    mesh = Mesh(np.asarray(devices), ("core",))
    in_specs = (PartitionSpec("core"),) * (n_params + n_outs)
    out_specs = (PartitionSpec("core"),) * n_outs
    sharded = jax.jit(
        shard_map(_body, mesh=mesh, in_specs=in_specs,
                  out_specs=out_specs, check_rep=False),
        donate_argnums=donate,
        keep_unused=True,
    )
    runner = {
        "jit": sharded, "in_names": in_names, "out_names": out_names,
        "out_avals": out_avals, "zero_shapes": zero_shapes, "mesh": mesh,
    }
    _runner_cache[key] = runner
    return runner


def _run_cores(in_maps):
    runner = _get_runner()
    concat_in = [
        np.concatenate([in_maps[c][name] for c in range(NCORES)], axis=0)
        for name in runner["in_names"]
    ]
    concat_zeros = [
        np.zeros((NCORES * s[0], *s[1:]), d) for (s, d) in runner["zero_shapes"]
    ]
    out_arrs = runner["jit"](*concat_in, *concat_zeros)
    results = []
    for c in range(NCORES):
        results.append({
            name: np.asarray(out_arrs[i]).reshape(
                NCORES, *runner["out_avals"][i].shape)[c]
            for i, name in enumerate(runner["out_names"])
        })
    return results


def benchmark(in_maps, iters=16):
    """Estimate device exec time: pre-stage inputs and `iters` donated
    zero-output sets on device, then enqueue all executions back-to-back
    and block once -- per-call slope approximates device exec time."""
    import time as _time
    import jax
    from jax.sharding import NamedSharding, PartitionSpec
    runner = _get_runner()
    mesh = runner["mesh"]
    shard = NamedSharding(mesh, PartitionSpec("core"))
    concat_in = [
        np.concatenate([in_maps[c][name] for c in range(NCORES)], axis=0)
        for name in runner["in_names"]
    ]
    dev_in = [jax.device_put(a, shard) for a in concat_in]
    fn = runner["jit"]

    def zeros_dev():
        return [
            jax.device_put(np.zeros((NCORES * s[0], *s[1:]), d), shard)
            for (s, d) in runner["zero_shapes"]
        ]

    out = fn(*dev_in, *zeros_dev())
    jax.block_until_ready(out)
    # pre-stage all zero sets (donated per call)
    zsets = [zeros_dev() for _ in range(iters)]
    jax.block_until_ready(zsets)
    outs = []
    t0 = _time.perf_counter()
    for z in zsets:
        outs.append(fn(*dev_in, *z))
    jax.block_until_ready(outs)
    t1 = _time.perf_counter()
    per_call = (t1 - t0) / iters
    # serial (blocking) timing for comparison
    z = zeros_dev()
    jax.block_until_ready(z)
    t2 = _time.perf_counter()
    out = fn(*dev_in, *z)
    jax.block_until_ready(out)
    t3 = _time.perf_counter()
    return per_call, (t3 - t2)


def _np_in_dt():
    if MM_MODE == "bf16":
        import ml_dtypes
        return ml_dtypes.bfloat16
    return np.float32


def _prep_core_inputs(Q, K, sketching_matrix, random_sign):
    """Host-side shard prep: per core one packed [128, INW] array."""
    X = np.concatenate([np.asarray(Q, np.float32),
                        np.asarray(K, np.float32)], axis=2)  # [B,H,4096,64]
    sk = np.asarray(sketching_matrix).astype(np.int64)       # [B, M, D]
    sign = np.asarray(random_sign, dtype=np.float32)         # [M, D]

    # sign-delta weight matrix W[32m+dl, 32c+dl] = sign[m, 32c+dl]
    W = np.zeros((128, D), dtype=np.float32)
    for m in range(M):
        for c in range(D // 32):
            dl = np.arange(32)
            W[32 * m + dl, 32 * c + dl] = sign[m, 32 * c + dl]

    np_dt = _np_in_dt()
    in_maps = []
    for core in range(NCORES):
        packed = np.empty((128, INW), dtype=np.float32)
        for pr in range(PAIRS):
            pair = core * PAIRS + pr
            b, h = divmod(pair, H)
            Xp = X[b, h]                            # [4096, 64]
            packed[64 * pr:64 * (pr + 1), 0:SEQ2] = Xp.T
            lm = Xp[sk[b]]                          # [M, D, 64]
            # landmark order l' = 128c + 32m + dl where d = 32c + dl
            lmp = lm.reshape(M, D // 32, 32, P).transpose(1, 0, 2, 3)
            lmp = lmp.reshape(L, P)                 # [(c, m, dl), 64]
            packed[64 * pr:64 * (pr + 1), SEQ2:SEQ2 + L] = lmp.T
        packed[:, SEQ2 + L:INW] = W
        in_maps.append({"inp": packed.astype(np_dt)})
    return in_maps


def kernel(Q, K, sketching_matrix, random_sign):
    in_maps = _prep_core_inputs(Q, K, sketching_matrix, random_sign)
    results = _run_cores(in_maps)
    # unshard: device out [PAIRS, 256, 4096] (d-major) -> [B, H, 4096, 256]
    AS = np.empty((B, H, SEQ2, D), dtype=np.float32)
    for core in range(NCORES):
        o = results[core]["out"]                # [PAIRS, 256, 4096]
        for pr in range(PAIRS):
            pair = core * PAIRS + pr
            b, h = divmod(pair, H)
            AS[b, h] = o[pr].T
    return AS



# revision 3
# speedup vs baseline: 34.9953x; 34.9953x over previous
"""Trainium2 Bass kernel for the sketched-attention RS_SM op.

Reference semantics (per (b,h) pair):
    X  = concat([Q, K], axis=seq)                      # [4096, 64]
    XS = gather of 1024 landmark rows of X             # [m=4, d=256, 64]
    AS[n, d] = sum_m sign[m, d] * exp(X[n] . XS[m, d]) # [4096, 256]

Sharding: 16 (b,h) pairs over 8 cores = 2 pairs/core, no cross-core comms.

The kernel is exp-throughput bound: 8.4M exps/core.  ScalarE (the only
engine with a real exp) does 1 elem/lane/cycle @1.2GHz = ~55us for all of
them, so the work is split: chunks 0-5 of each block go through ScalarE
activation, chunks 6-7 through a Schraudolph-style fast exp on VectorE
(one tensor_scalar op: y_bits = int16(x * 128/ln2 + (16256 - 4.5)), the
int16 bit pattern IS the bf16 approximation of exp(x); ~1.6% rms on 25%
of the output columns, well inside the 2e-2 gate).

Device pipeline per block (t-chunk of tokens, pair):
  MM1  : TensorE, lhsT = landmarksT [64, 128] (stationary), rhs = X^T
         -> PSUM [128 lmk, n].  Pairs row-tiled (K=64 each).
  exp  : ScalarE activation PSUM -> SBUF bf16 (chunks 0-5, 2 insts),
         VectorE Schraudolph PSUM -> SBUF int16-as-bf16 (chunks 6-7).
  MM2  : TensorE, lhsT = sign-delta W [128, 32], rhs = exp tile [128, n]
         -> PSUM [32 d, n], 8 col-tiled into one [128, 2n] tile.
         Signed m-reduction on the TensorE.  MM2 for block k is issued
         interleaved with MM1 of block k+1 so the PE's in-order queue
         never makes ScalarE/VectorE wait behind MM2.
  copy : one VectorE PSUM -> SBUF bf16 cast per block, DMA out
         [pair, 256, 4096] (d-major).

Landmark order is permuted (host-side) so chunk c holds (m, dl) for
d = 32c + dl: partition p = 32*m + dl.  W[32m+dl, 32c+dl] = sign[m, 32c+dl].
Host transposes the [256, 4096] device output to [4096, 256] at unshard.

Startup: a dummy 2-element activation at t=0 pulls the exp table load
(~1.3us) under the input DMAs; the packed input [128, 5376] (X^T|lmkT|W)
is DMA'd in 6 pieces ordered so block 0's operands land first.  The last
block is split into two 256-token sub-blocks to shorten the drain tail.

_build_nc(loop_n=R) wraps the body in a tc.For_i hardware loop; test.py
runs two R values and takes the slope to measure per-iteration device
time with host dispatch overhead cancelled.
"""

import os
import sys
import types
from contextlib import ExitStack

import numpy as np

sys.path.insert(0, "/opt/trn_rl_repo")

# The axon client in this container lacks the NTFF profile hook module;
# provide a stub so bass_utils' trace path degrades gracefully.
try:
    import antenv.axon_hooks  # noqa: F401
except ImportError:
    _stub = types.ModuleType("antenv.axon_hooks")
    _stub.get_axon_ntff_profile_hook = lambda: None
    sys.modules["antenv.axon_hooks"] = _stub

import concourse.bacc as bacc
import concourse.bass as bass
import concourse.mybir as mybir
import concourse.tile as tile

B, H, N, P = 2, 8, 2048, 64
M, D = 4, 256
SEQ2 = 2 * N                      # 4096 tokens per pair
NCORES = 8
PAIRS = (B * H) // NCORES         # 2 pairs per core
L = M * D                         # 1024 landmarks per pair
TCH = 512                         # token chunk (matmul moving dim)
NT = SEQ2 // TCH                  # 8 token chunks
INW = SEQ2 + L + D                # packed input width: xt | lt | w
F32 = mybir.dt.float32
F32R = mybir.dt.float32r
BF16 = mybir.dt.bfloat16
I16 = mybir.dt.int16

# "bf16": cast inputs to bf16; "f32r": fp32 data through the PE fast path.
MM_MODE = os.environ.get("KERNEL_MM_MODE", "bf16")
OUT_BF16 = os.environ.get("KERNEL_OUT_BF16", "1") == "1"
# chunks 6,7 of each block take the VectorE Schraudolph exp (bf16 mode only)
DVE_EXP = os.environ.get("KERNEL_DVE_EXP", "1") == "1" and MM_MODE == "bf16"

SCHRAU_A = float(2.0 ** 7 / np.log(2.0))
SCHRAU_C = float(os.environ.get("KERNEL_SCHRAU_C", "4.5"))
SCHRAU_B = 16256.0 - SCHRAU_C

_nc_cache = {}


def _build_nc(loop_n=1):
    nc = bacc.Bacc(
        "TRN2", target_bir_lowering=False, debug=False, num_devices=NCORES,
    )
    in_dt = BF16 if MM_MODE == "bf16" else F32R
    out_dt = BF16 if OUT_BF16 else F32

    inp = nc.dram_tensor("inp", [128, INW], in_dt, kind="ExternalInput")
    out = nc.dram_tensor("out", [PAIRS, D, SEQ2], out_dt, kind="ExternalOutput")

    with tile.TileContext(nc) as tc, ExitStack() as ctx:
        const_pool = ctx.enter_context(tc.tile_pool(name="const", bufs=1))
        eps_pool = ctx.enter_context(tc.tile_pool(name="eps", bufs=2, space="PSUM"))
        asps_pool = ctx.enter_context(tc.tile_pool(name="asps", bufs=1, space="PSUM"))
        esb_pool = ctx.enter_context(tc.tile_pool(name="esb", bufs=3))
        assb_pool = ctx.enter_context(tc.tile_pool(name="assb", bufs=3))
        warm_pool = ctx.enter_context(tc.tile_pool(name="warm", bufs=1))

        # blocks: (pr, t, off, ln); last 512-token block split into two 256s
        blocks = []
        for pr in range(PAIRS):
            for t in range(NT):
                if pr == PAIRS - 1 and t == NT - 1:
                    blocks.append((pr, t, 0, TCH // 2))
                    blocks.append((pr, t, TCH // 2, TCH // 2))
                else:
                    blocks.append((pr, t, 0, TCH))

        def issue_mm2_half(w_sb, prev, half, as_ps):
            e_sb, pr, t, off, ln = prev
            ecs = e_sb[:].rearrange("p (c n) -> p c n", c=8)
            for j in range(4):
                c = 4 * half + j
                nc.tensor.matmul(
                    as_ps[32 * j:32 * (j + 1), half * ln:(half + 1) * ln],
                    lhsT=w_sb[:, 32 * c:32 * (c + 1)],
                    rhs=ecs[:, c, off:off + ln],
                    start=True, stop=True,
                    tile_position=(0, 32 * j),
                )

        def finish_block(prev, as_ps, as_sb):
            e_sb, pr, t, off, ln = prev
            nc.vector.tensor_copy(as_sb[:, :2 * ln], as_ps[:, :2 * ln])
            hbm = out[pr].rearrange("(h p) n -> p h n", p=128)[
                :, :, t * TCH + off:t * TCH + off + ln
            ]
            sb = as_sb[:, :2 * ln].rearrange("p (h n) -> p h n", h=2)
            nc.sync.dma_start(hbm, sb)

        def body():
            # dummy activation first: pulls the exp table load under the DMAs
            warm = warm_pool.tile([128, 4], F32, tag="warm")
            nc.vector.memset(warm[:, 0:2], 0.0)
            nc.scalar.activation(
                warm[:, 2:4], warm[:, 0:2], mybir.ActivationFunctionType.Exp,
            )

            inp_sb = const_pool.tile([128, INW], in_dt, tag="inp_sb")
            # pair-0 operands first, then W, then pair-1
            nc.sync.dma_start(inp_sb[0:64, SEQ2:SEQ2 + L], inp[0:64, SEQ2:SEQ2 + L])
            nc.sync.dma_start(inp_sb[0:64, 0:TCH], inp[0:64, 0:TCH])
            nc.sync.dma_start(inp_sb[0:64, TCH:SEQ2], inp[0:64, TCH:SEQ2])
            nc.sync.dma_start(inp_sb[:, SEQ2 + L:INW], inp[:, SEQ2 + L:INW])
            nc.sync.dma_start(inp_sb[64:128, SEQ2:SEQ2 + L],
                              inp[64:128, SEQ2:SEQ2 + L])
            nc.sync.dma_start(inp_sb[64:128, 0:SEQ2], inp[64:128, 0:SEQ2])
            xt_sb = inp_sb[:, 0:SEQ2]
            lt_sb = inp_sb[:, SEQ2:SEQ2 + L]
            w_sb = inp_sb[:, SEQ2 + L:INW]

            prev = None          # (e_sb, pr, t, off, ln)
            prev_ps = None       # (as_ps, as_sb) of prev

            for pr, t, off, ln in blocks:
                rows = slice(64 * pr, 64 * (pr + 1))
                rhs_x = xt_sb[rows, t * TCH + off:t * TCH + off + ln]
                e_sb = esb_pool.tile([128, 8 * TCH], in_dt, tag="esb")
                ecs = e_sb[:].rearrange("p (c n) -> p c n", c=8)
                groups = [(0, 1, 2), (3, 4, 5), (6, 7)]
                for gidx, grp in enumerate(groups):
                    e_ps = eps_pool.tile([128, TCH * 3], F32, tag="eps")
                    for gi, c in enumerate(grp):
                        nc.tensor.matmul(
                            e_ps[:, gi * ln:(gi + 1) * ln],
                            lhsT=lt_sb[rows, 128 * c:128 * (c + 1)],
                            rhs=rhs_x,
                            start=True, stop=True,
                            tile_position=(64 * pr, 0),
                        )
                    e_out = ecs[:, grp[0]:grp[-1] + 1, off:off + ln]
                    if gidx == 2 and DVE_EXP:
                        nc.vector.tensor_scalar(
                            e_out.bitcast(I16),
                            e_ps[:, :2 * ln],
                            SCHRAU_A, SCHRAU_B,
                            mybir.AluOpType.mult, mybir.AluOpType.add,
                        )
                    else:
                        nc.scalar.activation(
                            e_out, e_ps[:, :len(grp) * ln],
                            mybir.ActivationFunctionType.Exp,
                        )
                    # previous block's MM2 halves slot between MM1 groups so
                    # the PE queue stays ahead of the exp engines
                    if prev is not None and gidx in (1, 2):
                        if gidx == 1:
                            as_ps = asps_pool.tile([128, 2 * TCH], F32, tag="asps")
                            as_sb = assb_pool.tile([128, 2 * TCH], out_dt, tag="assb")
                            prev_ps = (as_ps, as_sb)
                        issue_mm2_half(w_sb, prev, gidx - 1, prev_ps[0])
                if prev is not None:
                    finish_block(prev, *prev_ps)
                prev = (e_sb, pr, t, off, ln)

            # drain: last block's MM2 + copy + DMA
            as_ps = asps_pool.tile([128, 2 * TCH], F32, tag="asps")
            as_sb = assb_pool.tile([128, 2 * TCH], out_dt, tag="assb")
            for half in range(2):
                issue_mm2_half(w_sb, prev, half, as_ps)
            finish_block(prev, as_ps, as_sb)

        if loop_n > 1:
            with tc.For_i(0, loop_n):
                body()
        else:
            body()
    nc.compile()
    return nc


def _get_nc(loop_n=1):
    key = (MM_MODE, OUT_BF16, DVE_EXP, loop_n)
    if key not in _nc_cache:
        _nc_cache[key] = _build_nc(loop_n)
    return _nc_cache[key]


_runner_cache = {}


def _get_runner(loop_n=1):
    """Build (once) a jitted shard_map callable over the 8 cores, mirroring
    bass2jax.run_bass_via_pjrt but cached so repeat calls don't re-trace."""
    key = (MM_MODE, OUT_BF16, DVE_EXP, loop_n)
    if key in _runner_cache:
        return _runner_cache[key]
    import jax
    from jax.sharding import Mesh, PartitionSpec
    try:
        from jax.experimental.shard_map import shard_map
    except ImportError:
        from jax.shard_map import shard_map  # newer jax
    from concourse import bass2jax as b2j

    b2j.install_neuronx_cc_hook()
    nc = _get_nc(loop_n)

    partition_name = (
        nc.partition_id_tensor.name if nc.partition_id_tensor else None
    )
    in_names, out_names, out_avals, zero_shapes = [], [], [], []
    for alloc in nc.m.functions[0].allocations:
        if not isinstance(alloc, mybir.MemoryLocationSet):
            continue
        name = alloc.memorylocations[0].name
        if alloc.kind == "ExternalInput":
            if name != partition_name:
                in_names.append(name)
        elif alloc.kind == "ExternalOutput":
            out_names.append(name)
            shape = tuple(alloc.tensor_shape)
            dtype = mybir.dt.np(alloc.dtype)
            out_avals.append(jax.core.ShapedArray(shape, dtype))
            zero_shapes.append((shape, dtype))
    n_params = len(in_names)
    n_outs = len(out_avals)
    all_names = list(in_names) + list(out_names)
    if partition_name is not None:
        all_names.append(partition_name)
    donate = tuple(range(n_params, n_params + n_outs))

    def _body(*args):
        operands = list(args)
        if partition_name is not None:
            operands.append(b2j.partition_id_tensor())
        outs = b2j._bass_exec_p.bind(
            *operands,
            out_avals=tuple(out_avals),
            in_names=tuple(all_names),
            out_names=tuple(out_names),
            lowering_input_output_aliases=(),
            sim_require_finite=True,
            sim_require_nnan=True,
            nc=nc,
        )
        return tuple(outs)

    devices = jax.devices()[:NCORES]
    mesh = Mesh(np.asarray(devices), ("core",))
    in_specs = (PartitionSpec("core"),) * (n_params + n_outs)
    out_specs = (PartitionSpec("core"),) * n_outs
    sharded = jax.jit(
        shard_map(_body, mesh=mesh, in_specs=in_specs,
                  out_specs=out_specs, check_rep=False),
        donate_argnums=donate,
        keep_unused=True,
    )
    runner = {
        "jit": sharded, "in_names": in_names, "out_names": out_names,
        "out_avals": out_avals, "zero_shapes": zero_shapes, "mesh": mesh,
    }
    _runner_cache[key] = runner
    return runner


def _run_cores(in_maps):
    runner = _get_runner()
    concat_in = [
        np.concatenate([in_maps[c][name] for c in range(NCORES)], axis=0)
        for name in runner["in_names"]
    ]
    concat_zeros = [
        np.zeros((NCORES * s[0], *s[1:]), d) for (s, d) in runner["zero_shapes"]
    ]
    out_arrs = runner["jit"](*concat_in, *concat_zeros)
    results = []
    for c in range(NCORES):
        results.append({
            name: np.asarray(out_arrs[i]).reshape(
                NCORES, *runner["out_avals"][i].shape)[c]
            for i, name in enumerate(runner["out_names"])
        })
    return results


def _bench_one(in_maps, loop_n, iters):
    """Pipelined per-call time for the loop_n-iteration NEFF."""
    import time as _time
    import jax
    from jax.sharding import NamedSharding, PartitionSpec
    runner = _get_runner(loop_n)
    mesh = runner["mesh"]
    shard = NamedSharding(mesh, PartitionSpec("core"))
    concat_in = [
        np.concatenate([in_maps[c][name] for c in range(NCORES)], axis=0)
        for name in runner["in_names"]
    ]
    dev_in = [jax.device_put(a, shard) for a in concat_in]
    fn = runner["jit"]

    def zeros_dev():
        return [
            jax.device_put(np.zeros((NCORES * s[0], *s[1:]), d), shard)
            for (s, d) in runner["zero_shapes"]
        ]

    out = fn(*dev_in, *zeros_dev())
    jax.block_until_ready(out)
    best = None
    for _rep in range(3):
        zsets = [zeros_dev() for _ in range(iters)]
        jax.block_until_ready(zsets)
        outs = []
        t0 = _time.perf_counter()
        for z in zsets:
            outs.append(fn(*dev_in, *z))
        jax.block_until_ready(outs)
        t1 = _time.perf_counter()
        per_call = (t1 - t0) / iters
        best = per_call if best is None else min(best, per_call)
    return best


def benchmark_device(in_maps, r_small=32, r_big=256, iters=8):
    """Per-iteration device exec time via the two-point slope of in-NEFF
    hardware loops: (T(r_big) - T(r_small)) / (r_big - r_small).  Constant
    per-call dispatch overhead (axon RTT, jax dispatch) cancels in the
    difference; each T is itself the min-of-3 pipelined per-call slope."""
    t_small = _bench_one(in_maps, r_small, iters)
    t_big = _bench_one(in_maps, r_big, iters)
    t_iter = (t_big - t_small) / (r_big - r_small)
    return t_iter, t_small, t_big


def benchmark(in_maps, iters=16):
    """Legacy single-shot estimate: pipelined per-call slope (includes host
    dispatch; see benchmark_device for the amortized device-only number)."""
    import time as _time
    import jax
    from jax.sharding import NamedSharding, PartitionSpec
    runner = _get_runner()
    mesh = runner["mesh"]
    shard = NamedSharding(mesh, PartitionSpec("core"))
    concat_in = [
        np.concatenate([in_maps[c][name] for c in range(NCORES)], axis=0)
        for name in runner["in_names"]
    ]
    dev_in = [jax.device_put(a, shard) for a in concat_in]
    fn = runner["jit"]

    def zeros_dev():
        return [
            jax.device_put(np.zeros((NCORES * s[0], *s[1:]), d), shard)
            for (s, d) in runner["zero_shapes"]
        ]

    out = fn(*dev_in, *zeros_dev())
    jax.block_until_ready(out)
    zsets = [zeros_dev() for _ in range(iters)]
    jax.block_until_ready(zsets)
    outs = []
    t0 = _time.perf_counter()
    for z in zsets:
        outs.append(fn(*dev_in, *z))
    jax.block_until_ready(outs)
    t1 = _time.perf_counter()
    per_call = (t1 - t0) / iters
    z = zeros_dev()
    jax.block_until_ready(z)
    t2 = _time.perf_counter()
    out = fn(*dev_in, *z)
    jax.block_until_ready(out)
    t3 = _time.perf_counter()
    return per_call, (t3 - t2)


def _np_in_dt():
    if MM_MODE == "bf16":
        import ml_dtypes
        return ml_dtypes.bfloat16
    return np.float32


def _prep_core_inputs(Q, K, sketching_matrix, random_sign):
    """Host-side shard prep: per core one packed [128, INW] array."""
    X = np.concatenate([np.asarray(Q, np.float32),
                        np.asarray(K, np.float32)], axis=2)  # [B,H,4096,64]
    sk = np.asarray(sketching_matrix).astype(np.int64)       # [B, M, D]
    sign = np.asarray(random_sign, dtype=np.float32)         # [M, D]

    # sign-delta weight matrix W[32m+dl, 32c+dl] = sign[m, 32c+dl]
    W = np.zeros((128, D), dtype=np.float32)
    for m in range(M):
        for c in range(D // 32):
            dl = np.arange(32)
            W[32 * m + dl, 32 * c + dl] = sign[m, 32 * c + dl]

    np_dt = _np_in_dt()
    in_maps = []
    for core in range(NCORES):
        packed = np.empty((128, INW), dtype=np.float32)
        for pr in range(PAIRS):
            pair = core * PAIRS + pr
            b, h = divmod(pair, H)
            Xp = X[b, h]                            # [4096, 64]
            packed[64 * pr:64 * (pr + 1), 0:SEQ2] = Xp.T
            lm = Xp[sk[b]]                          # [M, D, 64]
            # landmark order l' = 128c + 32m + dl where d = 32c + dl
            lmp = lm.reshape(M, D // 32, 32, P).transpose(1, 0, 2, 3)
            lmp = lmp.reshape(L, P)                 # [(c, m, dl), 64]
            packed[64 * pr:64 * (pr + 1), SEQ2:SEQ2 + L] = lmp.T
        packed[:, SEQ2 + L:INW] = W
        in_maps.append({"inp": packed.astype(np_dt)})
    return in_maps


def kernel(Q, K, sketching_matrix, random_sign):
    in_maps = _prep_core_inputs(Q, K, sketching_matrix, random_sign)
    results = _run_cores(in_maps)
    # unshard: device out [PAIRS, 256, 4096] (d-major) -> [B, H, 4096, 256]
    AS = np.empty((B, H, SEQ2, D), dtype=np.float32)
    for core in range(NCORES):
        o = results[core]["out"]                # [PAIRS, 256, 4096]
        for pr in range(PAIRS):
            pair = core * PAIRS + pr
            b, h = divmod(pair, H)
            AS[b, h] = o[pr].T.astype(np.float32)
    return AS


# revision 16
# speedup vs baseline: 38.9207x; 1.1122x over previous
"""Trainium2 Bass kernel for the sketched-attention RS_SM op.

Reference semantics (per (b,h) pair):
    X  = concat([Q, K], axis=seq)                      # [4096, 64]
    XS = gather of 1024 landmark rows of X             # [m=4, d=256, 64]
    AS[n, d] = sum_m sign[m, d] * exp(X[n] . XS[m, d]) # [4096, 256]

Sharding: 16 (b,h) pairs over 8 cores = 2 pairs/core, no cross-core comms.

The kernel is exp-throughput bound: 8.4M exps/core.  ScalarE (the only
engine with a real exp) does 1 elem/lane/cycle @1.2GHz = ~55us for all of
them, so the work is split: chunks 0-5 of each block go through ScalarE
activation, chunks 6-7 through a Schraudolph-style fast exp on VectorE
(one tensor_scalar op: y_bits = int16(x * 128/ln2 + (16256 - 4.5)), the
int16 bit pattern IS the bf16 approximation of exp(x); ~1.6% rms on 25%
of the output columns, well inside the 2e-2 gate).

Device pipeline per block (t-chunk of tokens, pair):
  MM1  : TensorE, lhsT = landmarksT [64, 128] (stationary), rhs = X^T
         -> PSUM [128 lmk, n].  Pairs row-tiled (K=64 each).
  exp  : ScalarE activation PSUM -> SBUF bf16 (chunks 0-5, 2 insts),
         VectorE Schraudolph PSUM -> SBUF int16-as-bf16 (chunks 6-7).
  MM2  : TensorE, lhsT = sign-delta W [128, 32], rhs = exp tile [128, n]
         -> PSUM [32 d, n], 8 col-tiled into one [128, 2n] tile.
         Signed m-reduction on the TensorE.  MM2 for block k is issued
         interleaved with MM1 of block k+1 so the PE's in-order queue
         never makes ScalarE/VectorE wait behind MM2.
  copy : one VectorE PSUM -> SBUF bf16 cast per block, DMA out
         [pair, 256, 4096] (d-major).

Landmark order is permuted (host-side) so chunk c holds (m, dl) for
d = 32c + dl: partition p = 32*m + dl.  W[32m+dl, 32c+dl] = sign[m, 32c+dl].
Host transposes the [256, 4096] device output to [4096, 256] at unshard.

Startup: a dummy 2-element activation at t=0 pulls the exp table load
(~1.3us) under the input DMAs; the packed input [128, 5376] (X^T|lmkT|W)
is DMA'd in 6 pieces ordered so block 0's operands land first.  The last
block is split into two 256-token sub-blocks to shorten the drain tail.

_build_nc(loop_n=R) wraps the body in a tc.For_i hardware loop; test.py
runs two R values and takes the slope to measure per-iteration device
time with host dispatch overhead cancelled.
"""

import os
import sys
import types
from contextlib import ExitStack

import numpy as np

sys.path.insert(0, "/opt/trn_rl_repo")

# The axon client in this container lacks the NTFF profile hook module;
# provide a stub so bass_utils' trace path degrades gracefully.
try:
    import antenv.axon_hooks  # noqa: F401
except ImportError:
    _stub = types.ModuleType("antenv.axon_hooks")
    _stub.get_axon_ntff_profile_hook = lambda: None
    sys.modules["antenv.axon_hooks"] = _stub

import concourse.bacc as bacc
import concourse.bass as bass
import concourse.mybir as mybir
import concourse.tile as tile

B, H, N, P = 2, 8, 2048, 64
M, D = 4, 256
SEQ2 = 2 * N                      # 4096 tokens per pair
NCORES = 8
PAIRS = (B * H) // NCORES         # 2 pairs per core
L = M * D                         # 1024 landmarks per pair
TCH = 512                         # token chunk (matmul moving dim)
NT = SEQ2 // TCH                  # 8 token chunks
# packed input width: xt(pair0)|xt(pair1)|lt(pair0)|lt(pair1)|w.  X^T and
# landmarksT are duplicated into both 64-partition halves so consecutive MM1
# chunks can alternate PE row groups (LDWEIGHTS for one row group pulls ahead
# of the in-flight matmul on the other; same-row-group LDWs serialize).
INW = 2 * SEQ2 + 2 * L + D
XT_OFF = 0                        # + pr * SEQ2
LT_OFF = 2 * SEQ2                 # + pr * L
W_OFF = 2 * SEQ2 + 2 * L
F32 = mybir.dt.float32
F32R = mybir.dt.float32r
BF16 = mybir.dt.bfloat16
I16 = mybir.dt.int16

# "bf16": cast inputs to bf16; "f32r": fp32 data through the PE fast path.
MM_MODE = os.environ.get("KERNEL_MM_MODE", "bf16")
OUT_BF16 = os.environ.get("KERNEL_OUT_BF16", "1") == "1"
# chunks 6,7 of each block take the VectorE Schraudolph exp (bf16 mode only)
DVE_EXP = os.environ.get("KERNEL_DVE_EXP", "1") == "1" and MM_MODE == "bf16"

SCHRAU_A = float(2.0 ** 7 / np.log(2.0))
SCHRAU_C = float(os.environ.get("KERNEL_SCHRAU_C", "4.5"))
SCHRAU_B = 16256.0 - SCHRAU_C

_nc_cache = {}


def _build_nc(loop_n=1, _parts="full"):
    # _parts: benchmarking aid — "full" (default), "dma" (input DMA only),
    # "mm1" (+MM1), "exp" (+exp engines), "mm2" (+MM2/copy, no out-DMA).
    nc = bacc.Bacc(
        "TRN2", target_bir_lowering=False, debug=False, num_devices=NCORES,
    )
    in_dt = BF16 if MM_MODE == "bf16" else F32R
    out_dt = BF16 if OUT_BF16 else F32

    inp = nc.dram_tensor("inp", [128, INW], in_dt, kind="ExternalInput")
    out = nc.dram_tensor("out", [PAIRS, D, SEQ2], out_dt, kind="ExternalOutput")

    with tile.TileContext(nc) as tc, ExitStack() as ctx:
        const_pool = ctx.enter_context(tc.tile_pool(name="const", bufs=1))
        eps_pool = ctx.enter_context(tc.tile_pool(name="eps", bufs=2, space="PSUM"))
        asps_pool = ctx.enter_context(tc.tile_pool(name="asps", bufs=1, space="PSUM"))
        esb_pool = ctx.enter_context(tc.tile_pool(name="esb", bufs=3))
        assb_pool = ctx.enter_context(tc.tile_pool(name="assb", bufs=3))
        warm_pool = ctx.enter_context(tc.tile_pool(name="warm", bufs=1))

        # blocks: (pr, t, off, ln); last 512-token block split into two 256s
        blocks = []
        for pr in range(PAIRS):
            for t in range(NT):
                if pr == PAIRS - 1 and t == NT - 1:
                    blocks.append((pr, t, 0, TCH // 2))
                    blocks.append((pr, t, TCH // 2, TCH // 2))
                else:
                    blocks.append((pr, t, 0, TCH))

        def issue_mm2_half(w_sb, prev, half, as_ps):
            e_sb, pr, t, off, ln = prev
            ecs = e_sb[:].rearrange("p (c n) -> p c n", c=8)
            for j in range(4):
                c = 4 * half + j
                nc.tensor.matmul(
                    as_ps[32 * j:32 * (j + 1), half * ln:(half + 1) * ln],
                    lhsT=w_sb[:, 32 * c:32 * (c + 1)],
                    rhs=ecs[:, c, off:off + ln],
                    start=True, stop=True,
                    tile_position=(0, 32 * j),
                )

        def dma_out_block(prev, as_sb):
            e_sb, pr, t, off, ln = prev
            hbm = out[pr].rearrange("(h p) n -> p h n", p=128)[
                :, :, t * TCH + off:t * TCH + off + ln
            ]
            sb = as_sb[:, :2 * ln].rearrange("p (h n) -> p h n", h=2)
            nc.sync.dma_start(hbm, sb)

        def body():
            # dummy activation first: pulls the exp table load under the DMAs
            warm = warm_pool.tile([128, 4], F32, tag="warm")
            nc.vector.memset(warm[:, 0:2], 0.0)
            nc.scalar.activation(
                warm[:, 2:4], warm[:, 0:2], mybir.ActivationFunctionType.Exp,
            )

            inp_sb = const_pool.tile([128, INW], in_dt, tag="inp_sb")
            # pair-0 operands first, then W, then pair-1
            nc.sync.dma_start(inp_sb[:, LT_OFF:LT_OFF + L],
                              inp[:, LT_OFF:LT_OFF + L])
            nc.sync.dma_start(inp_sb[:, XT_OFF:XT_OFF + TCH],
                              inp[:, XT_OFF:XT_OFF + TCH])
            nc.sync.dma_start(inp_sb[:, XT_OFF + TCH:XT_OFF + SEQ2],
                              inp[:, XT_OFF + TCH:XT_OFF + SEQ2])
            nc.sync.dma_start(inp_sb[:, W_OFF:INW], inp[:, W_OFF:INW])
            nc.sync.dma_start(inp_sb[:, LT_OFF + L:LT_OFF + 2 * L],
                              inp[:, LT_OFF + L:LT_OFF + 2 * L])
            nc.sync.dma_start(inp_sb[:, XT_OFF + SEQ2:XT_OFF + 2 * SEQ2],
                              inp[:, XT_OFF + SEQ2:XT_OFF + 2 * SEQ2])
            w_sb = inp_sb[:, W_OFF:INW]

            if _parts == "dma":
                return
            do_exp = _parts in ("exp", "mm2", "full")
            do_mm2 = _parts in ("mm2", "full")
            do_out = _parts == "full"

            prev = None          # (e_sb, pr, t, off, ln)
            prev_ps = None       # (as_ps, as_sb) of prev

            for pr, t, off, ln in blocks:
                xt = inp_sb[:, XT_OFF + pr * SEQ2:XT_OFF + (pr + 1) * SEQ2]
                lt = inp_sb[:, LT_OFF + pr * L:LT_OFF + (pr + 1) * L]
                tok = slice(t * TCH + off, t * TCH + off + ln)
                e_sb = esb_pool.tile([128, 8 * TCH], in_dt, tag="esb")
                ecs = e_sb[:].rearrange("p (c n) -> p c n", c=8)
                groups = [(0, 1, 2), (3, 4, 5), (6, 7)]
                for gidx, grp in enumerate(groups):
                    e_ps = eps_pool.tile([128, TCH * 3], F32, tag="eps")
                    # chunk outputs at bank-aligned offsets (gi * TCH even for
                    # short sub-blocks): concurrent row-group matmuls must not
                    # write the same PSUM bank
                    epv = e_ps[:].rearrange("p (g n) -> p g n", g=3)
                    for gi, c in enumerate(grp):
                        half = c % 2
                        rows = slice(64 * half, 64 * (half + 1))
                        nc.tensor.matmul(
                            e_ps[:, gi * TCH:gi * TCH + ln],
                            lhsT=lt[rows, 128 * c:128 * (c + 1)],
                            rhs=xt[rows, tok],
                            start=True, stop=True,
                            tile_position=(64 * half, 0),
                        )
                    if not do_exp:
                        continue
                    e_out = ecs[:, grp[0]:grp[-1] + 1, off:off + ln]
                    if gidx == 2 and DVE_EXP:
                        nc.vector.tensor_scalar(
                            e_out.bitcast(I16),
                            epv[:, 0:2, 0:ln],
                            SCHRAU_A, SCHRAU_B,
                            mybir.AluOpType.mult, mybir.AluOpType.add,
                        )
                    else:
                        nc.scalar.activation(
                            e_out, epv[:, 0:len(grp), 0:ln],
                            mybir.ActivationFunctionType.Exp,
                        )
                    # previous block's MM2 halves slot between MM1 groups so
                    # the PE queue stays ahead of the exp engines
                    if do_mm2 and prev is not None and gidx in (1, 2):
                        if gidx == 1:
                            as_ps = asps_pool.tile([128, 2 * TCH], F32, tag="asps")
                            as_sb = assb_pool.tile([128, 2 * TCH], out_dt, tag="assb")
                            prev_ps = (as_ps, as_sb)
                        issue_mm2_half(w_sb, prev, gidx - 1, prev_ps[0])
                if do_mm2 and prev is not None:
                    nc.vector.tensor_copy(
                        prev_ps[1][:, :2 * prev[4]], prev_ps[0][:, :2 * prev[4]])
                    if do_out:
                        dma_out_block(prev, prev_ps[1])
                prev = (e_sb, pr, t, off, ln)

            if do_mm2:
                # drain: last block's MM2 + copy + DMA
                as_ps = asps_pool.tile([128, 2 * TCH], F32, tag="asps")
                as_sb = assb_pool.tile([128, 2 * TCH], out_dt, tag="assb")
                for half in range(2):
                    issue_mm2_half(w_sb, prev, half, as_ps)
                nc.vector.tensor_copy(as_sb[:, :2 * prev[4]], as_ps[:, :2 * prev[4]])
                if do_out:
                    dma_out_block(prev, as_sb)

        if loop_n > 1:
            with tc.For_i(0, loop_n):
                body()
        else:
            body()
    nc.compile()
    return nc


def _get_nc(loop_n=1, _parts="full"):
    key = (MM_MODE, OUT_BF16, DVE_EXP, loop_n, _parts)
    if key not in _nc_cache:
        _nc_cache[key] = _build_nc(loop_n, _parts)
    return _nc_cache[key]


_runner_cache = {}


def _get_runner(loop_n=1, _parts="full"):
    """Build (once) a jitted shard_map callable over the 8 cores, mirroring
    bass2jax.run_bass_via_pjrt but cached so repeat calls don't re-trace."""
    key = (MM_MODE, OUT_BF16, DVE_EXP, loop_n, _parts)
    if key in _runner_cache:
        return _runner_cache[key]
    import jax
    from jax.sharding import Mesh, PartitionSpec
    try:
        from jax.experimental.shard_map import shard_map
    except ImportError:
        from jax.shard_map import shard_map  # newer jax
    from concourse import bass2jax as b2j

    b2j.install_neuronx_cc_hook()
    nc = _get_nc(loop_n, _parts)

    partition_name = (
        nc.partition_id_tensor.name if nc.partition_id_tensor else None
    )
    in_names, out_names, out_avals, zero_shapes = [], [], [], []
    for alloc in nc.m.functions[0].allocations:
        if not isinstance(alloc, mybir.MemoryLocationSet):
            continue
        name = alloc.memorylocations[0].name
        if alloc.kind == "ExternalInput":
            if name != partition_name:
                in_names.append(name)
        elif alloc.kind == "ExternalOutput":
            out_names.append(name)
            shape = tuple(alloc.tensor_shape)
            dtype = mybir.dt.np(alloc.dtype)
            out_avals.append(jax.core.ShapedArray(shape, dtype))
            zero_shapes.append((shape, dtype))
    n_params = len(in_names)
    n_outs = len(out_avals)
    all_names = list(in_names) + list(out_names)
    if partition_name is not None:
        all_names.append(partition_name)
    donate = tuple(range(n_params, n_params + n_outs))

    def _body(*args):
        operands = list(args)
        if partition_name is not None:
            operands.append(b2j.partition_id_tensor())
        outs = b2j._bass_exec_p.bind(
            *operands,
            out_avals=tuple(out_avals),
            in_names=tuple(all_names),
            out_names=tuple(out_names),
            lowering_input_output_aliases=(),
            sim_require_finite=True,
            sim_require_nnan=True,
            nc=nc,
        )
        return tuple(outs)

    devices = jax.devices()[:NCORES]
    mesh = Mesh(np.asarray(devices), ("core",))
    in_specs = (PartitionSpec("core"),) * (n_params + n_outs)
    out_specs = (PartitionSpec("core"),) * n_outs
    sharded = jax.jit(
        shard_map(_body, mesh=mesh, in_specs=in_specs,
                  out_specs=out_specs, check_rep=False),
        donate_argnums=donate,
        keep_unused=True,
    )
    runner = {
        "jit": sharded, "in_names": in_names, "out_names": out_names,
        "out_avals": out_avals, "zero_shapes": zero_shapes, "mesh": mesh,
    }
    _runner_cache[key] = runner
    return runner


def _run_cores(in_maps):
    runner = _get_runner()
    concat_in = [
        np.concatenate([in_maps[c][name] for c in range(NCORES)], axis=0)
        for name in runner["in_names"]
    ]
    concat_zeros = [
        np.zeros((NCORES * s[0], *s[1:]), d) for (s, d) in runner["zero_shapes"]
    ]
    out_arrs = runner["jit"](*concat_in, *concat_zeros)
    results = []
    for c in range(NCORES):
        results.append({
            name: np.asarray(out_arrs[i]).reshape(
                NCORES, *runner["out_avals"][i].shape)[c]
            for i, name in enumerate(runner["out_names"])
        })
    return results


def _bench_setup(in_maps, loop_n, _parts="full"):
    import jax
    from jax.sharding import NamedSharding, PartitionSpec
    runner = _get_runner(loop_n, _parts)
    shard = NamedSharding(runner["mesh"], PartitionSpec("core"))
    concat_in = [
        np.concatenate([in_maps[c][name] for c in range(NCORES)], axis=0)
        for name in runner["in_names"]
    ]
    dev_in = [jax.device_put(a, shard) for a in concat_in]

    def zeros_dev():
        return [
            jax.device_put(np.zeros((NCORES * s[0], *s[1:]), d), shard)
            for (s, d) in runner["zero_shapes"]
        ]

    return runner["jit"], dev_in, zeros_dev


def _bench_round(fn, dev_in, zeros_dev, iters):
    import time as _time
    import jax
    zsets = [zeros_dev() for _ in range(iters)]
    jax.block_until_ready(zsets)
    t0 = _time.perf_counter()
    outs = [fn(*dev_in, *z) for z in zsets]
    jax.block_until_ready(outs)
    t1 = _time.perf_counter()
    return (t1 - t0) / iters


def benchmark_device(in_maps, r_small=128, r_big=1024, iters=4, rounds=4,
                     _parts="full"):
    """Per-iteration device exec time via the two-point slope of in-NEFF
    hardware loops: (T(r_big) - T(r_small)) / (r_big - r_small).  Per-call
    dispatch overhead (axon RTT, jax dispatch) cancels in the difference.
    The two loop sizes are measured in interleaved rounds so slow drift in
    dispatch overhead hits both equally; the median round slope is used."""
    fn_s, in_s, z_s = _bench_setup(in_maps, r_small, _parts)
    fn_b, in_b, z_b = _bench_setup(in_maps, r_big, _parts)
    # warm both
    _bench_round(fn_s, in_s, z_s, 1)
    _bench_round(fn_b, in_b, z_b, 1)
    slopes, t_smalls, t_bigs = [], [], []
    for _ in range(rounds):
        t_s = _bench_round(fn_s, in_s, z_s, iters)
        t_b = _bench_round(fn_b, in_b, z_b, iters)
        t_smalls.append(t_s)
        t_bigs.append(t_b)
        slopes.append((t_b - t_s) / (r_big - r_small))
    slopes.sort()
    t_iter = slopes[len(slopes) // 2]
    return t_iter, min(t_smalls), min(t_bigs)


def benchmark(in_maps, iters=16):
    """Legacy single-shot estimate: pipelined per-call slope (includes host
    dispatch; see benchmark_device for the amortized device-only number)."""
    import time as _time
    import jax
    from jax.sharding import NamedSharding, PartitionSpec
    runner = _get_runner()
    mesh = runner["mesh"]
    shard = NamedSharding(mesh, PartitionSpec("core"))
    concat_in = [
        np.concatenate([in_maps[c][name] for c in range(NCORES)], axis=0)
        for name in runner["in_names"]
    ]
    dev_in = [jax.device_put(a, shard) for a in concat_in]
    fn = runner["jit"]

    def zeros_dev():
        return [
            jax.device_put(np.zeros((NCORES * s[0], *s[1:]), d), shard)
            for (s, d) in runner["zero_shapes"]
        ]

    out = fn(*dev_in, *zeros_dev())
    jax.block_until_ready(out)
    zsets = [zeros_dev() for _ in range(iters)]
    jax.block_until_ready(zsets)
    outs = []
    t0 = _time.perf_counter()
    for z in zsets:
        outs.append(fn(*dev_in, *z))
    jax.block_until_ready(outs)
    t1 = _time.perf_counter()
    per_call = (t1 - t0) / iters
    z = zeros_dev()
    jax.block_until_ready(z)
    t2 = _time.perf_counter()
    out = fn(*dev_in, *z)
    jax.block_until_ready(out)
    t3 = _time.perf_counter()
    return per_call, (t3 - t2)


def _np_in_dt():
    if MM_MODE == "bf16":
        import ml_dtypes
        return ml_dtypes.bfloat16
    return np.float32


def _prep_core_inputs(Q, K, sketching_matrix, random_sign):
    """Host-side shard prep: per core one packed [128, INW] array."""
    X = np.concatenate([np.asarray(Q, np.float32),
                        np.asarray(K, np.float32)], axis=2)  # [B,H,4096,64]
    sk = np.asarray(sketching_matrix).astype(np.int64)       # [B, M, D]
    sign = np.asarray(random_sign, dtype=np.float32)         # [M, D]

    # sign-delta weight matrix W[32m+dl, 32c+dl] = sign[m, 32c+dl]
    W = np.zeros((128, D), dtype=np.float32)
    for m in range(M):
        for c in range(D // 32):
            dl = np.arange(32)
            W[32 * m + dl, 32 * c + dl] = sign[m, 32 * c + dl]

    np_dt = _np_in_dt()
    in_maps = []
    for core in range(NCORES):
        packed = np.empty((128, INW), dtype=np.float32)
        for pr in range(PAIRS):
            pair = core * PAIRS + pr
            b, h = divmod(pair, H)
            Xp = X[b, h]                            # [4096, 64]
            xts = slice(XT_OFF + pr * SEQ2, XT_OFF + (pr + 1) * SEQ2)
            packed[0:64, xts] = Xp.T
            packed[64:128, xts] = Xp.T              # dup for PE row group 1
            lm = Xp[sk[b]]                          # [M, D, 64]
            # landmark order l' = 128c + 32m + dl where d = 32c + dl
            lmp = lm.reshape(M, D // 32, 32, P).transpose(1, 0, 2, 3)
            lmp = lmp.reshape(L, P)                 # [(c, m, dl), 64]
            lts = slice(LT_OFF + pr * L, LT_OFF + (pr + 1) * L)
            packed[0:64, lts] = lmp.T
            packed[64:128, lts] = lmp.T
        packed[:, W_OFF:INW] = W
        in_maps.append({"inp": packed.astype(np_dt)})
    return in_maps


def kernel(Q, K, sketching_matrix, random_sign):
    in_maps = _prep_core_inputs(Q, K, sketching_matrix, random_sign)
    results = _run_cores(in_maps)
    # unshard: device out [PAIRS, 256, 4096] (d-major) -> [B, H, 4096, 256]
    AS = np.empty((B, H, SEQ2, D), dtype=np.float32)
    for core in range(NCORES):
        o = results[core]["out"]                # [PAIRS, 256, 4096]
        for pr in range(PAIRS):
            pair = core * PAIRS + pr
            b, h = divmod(pair, H)
            AS[b, h] = o[pr].T.astype(np.float32)
    return AS


# revision 19
# speedup vs baseline: 43.7442x; 1.1239x over previous
"""Trainium2 Bass kernel for the sketched-attention RS_SM op.

Reference semantics (per (b,h) pair):
    X  = concat([Q, K], axis=seq)                      # [4096, 64]
    XS = gather of 1024 landmark rows of X             # [m=4, d=256, 64]
    AS[n, d] = sum_m sign[m, d] * exp(X[n] . XS[m, d]) # [4096, 256]

Sharding: 16 (b,h) pairs over 8 cores = 2 pairs/core, no cross-core comms.

The kernel is exp-throughput bound: 8.4M exps/core.  ScalarE (the only
engine with a real exp) does 1 elem/lane/cycle @1.2GHz = ~55us for all of
them, so the work is split: chunks 0-5 of each block go through ScalarE
activation, chunks 6-7 through a Schraudolph-style fast exp on VectorE
(one tensor_scalar op: y_bits = int16(x * 128/ln2 + (16256 - 4.5)), the
int16 bit pattern IS the bf16 approximation of exp(x); ~1.6% rms on 25%
of the output columns, well inside the 2e-2 gate).

Device pipeline per block (t-chunk of tokens, pair):
  MM1  : TensorE, lhsT = landmarksT [64, 128] (stationary), rhs = X^T
         -> PSUM [128 lmk, n].  Pairs row-tiled (K=64 each).
  exp  : ScalarE activation PSUM -> SBUF bf16 (chunks 0-5, 2 insts),
         VectorE Schraudolph PSUM -> SBUF int16-as-bf16 (chunks 6-7).
  MM2  : TensorE, lhsT = sign-delta W [128, 32], rhs = exp tile [128, n]
         -> PSUM [32 d, n], 8 col-tiled into one [128, 2n] tile.
         Signed m-reduction on the TensorE.  MM2 for block k is issued
         interleaved with MM1 of block k+1 so the PE's in-order queue
         never makes ScalarE/VectorE wait behind MM2.
  copy : one VectorE PSUM -> SBUF bf16 cast per block, DMA out
         [pair, 256, 4096] (d-major).

Landmark order is permuted (host-side) so chunk c holds (m, dl) for
d = 32c + dl: partition p = 32*m + dl.  W[32m+dl, 32c+dl] = sign[m, 32c+dl].
Host transposes the [256, 4096] device output to [4096, 256] at unshard.

Startup: a dummy 2-element activation at t=0 pulls the exp table load
(~1.3us) under the input DMAs; the packed input [128, 5376] (X^T|lmkT|W)
is DMA'd in 6 pieces ordered so block 0's operands land first.  The last
block is split into two 256-token sub-blocks to shorten the drain tail.

_build_nc(loop_n=R) wraps the body in a tc.For_i hardware loop; test.py
runs two R values and takes the slope to measure per-iteration device
time with host dispatch overhead cancelled.
"""

import os
import sys
import types
from contextlib import ExitStack

import numpy as np

sys.path.insert(0, "/opt/trn_rl_repo")

# The axon client in this container lacks the NTFF profile hook module;
# provide a stub so bass_utils' trace path degrades gracefully.
try:
    import antenv.axon_hooks  # noqa: F401
except ImportError:
    _stub = types.ModuleType("antenv.axon_hooks")
    _stub.get_axon_ntff_profile_hook = lambda: None
    sys.modules["antenv.axon_hooks"] = _stub

import concourse.bacc as bacc
import concourse.bass as bass
import concourse.mybir as mybir
import concourse.tile as tile

B, H, N, P = 2, 8, 2048, 64
M, D = 4, 256
SEQ2 = 2 * N                      # 4096 tokens per pair
NCORES = 8
PAIRS = (B * H) // NCORES         # 2 pairs per core
L = M * D                         # 1024 landmarks per pair
TCH = 512                         # token chunk (matmul moving dim)
NT = SEQ2 // TCH                  # 8 token chunks
# packed input width: xt(pair0)|xt(pair1)|lt(pair0)|lt(pair1)|w.  X^T and
# landmarksT are duplicated into both 64-partition halves so consecutive MM1
# chunks can alternate PE row groups (LDWEIGHTS for one row group pulls ahead
# of the in-flight matmul on the other; same-row-group LDWs serialize).
INW = 2 * SEQ2 + 2 * L + D
XT_OFF = 0                        # + pr * SEQ2
LT_OFF = 2 * SEQ2                 # + pr * L
W_OFF = 2 * SEQ2 + 2 * L
F32 = mybir.dt.float32
F32R = mybir.dt.float32r
BF16 = mybir.dt.bfloat16
I16 = mybir.dt.int16

# "bf16": cast inputs to bf16; "f32r": fp32 data through the PE fast path.
MM_MODE = os.environ.get("KERNEL_MM_MODE", "bf16")
OUT_BF16 = os.environ.get("KERNEL_OUT_BF16", "1") == "1"
# chunks 6,7 of each block take the VectorE Schraudolph exp (bf16 mode only)
DVE_EXP = os.environ.get("KERNEL_DVE_EXP", "1") == "1" and MM_MODE == "bf16"

SCHRAU_A = float(2.0 ** 7 / np.log(2.0))
SCHRAU_C = float(os.environ.get("KERNEL_SCHRAU_C", "4.5"))
SCHRAU_B = 16256.0 - SCHRAU_C

_nc_cache = {}


def _build_nc(loop_n=1, _parts="full"):
    # _parts: benchmarking aid — "full" (default), "dma" (input DMA only),
    # "mm1" (+MM1), "exp" (+exp engines), "mm2" (+MM2/copy, no out-DMA).
    nc = bacc.Bacc(
        "TRN2", target_bir_lowering=False, debug=False, num_devices=NCORES,
    )
    in_dt = BF16 if MM_MODE == "bf16" else F32R
    out_dt = BF16 if OUT_BF16 else F32

    inp = nc.dram_tensor("inp", [128, INW], in_dt, kind="ExternalInput")
    out = nc.dram_tensor("out", [PAIRS, D, SEQ2], out_dt, kind="ExternalOutput")

    with tile.TileContext(nc) as tc, ExitStack() as ctx:
        const_pool = ctx.enter_context(tc.tile_pool(name="const", bufs=1))
        eps_pool = ctx.enter_context(tc.tile_pool(name="eps", bufs=2, space="PSUM"))
        # two 1-bank as_ps tiles: MM2 half k never waits on the previous
        # block's copy (which queues behind the DVE exp)
        asps_pool = ctx.enter_context(tc.tile_pool(name="asps", bufs=2, space="PSUM"))
        esb_pool = ctx.enter_context(tc.tile_pool(name="esb", bufs=3))
        assb_pool = ctx.enter_context(tc.tile_pool(name="assb", bufs=3))
        warm_pool = ctx.enter_context(tc.tile_pool(name="warm", bufs=1))

        # blocks: (pr, t, off, ln); last 512-token block split into two 256s
        blocks = []
        for pr in range(PAIRS):
            for t in range(NT):
                if pr == PAIRS - 1 and t == NT - 1:
                    blocks.append((pr, t, 0, TCH // 2))
                    blocks.append((pr, t, TCH // 2, TCH // 2))
                else:
                    blocks.append((pr, t, 0, TCH))

        def issue_mm2_half(w_sb, prev, half, as_sb):
            e_sb, pr, t, off, ln = prev
            ecs = e_sb[:].rearrange("p (c n) -> p c n", c=8)
            as_ps = asps_pool.tile([128, TCH], F32, tag="asps")
            for j in range(4):
                c = 4 * half + j
                nc.tensor.matmul(
                    as_ps[32 * j:32 * (j + 1), 0:ln],
                    lhsT=w_sb[:, 32 * c:32 * (c + 1)],
                    rhs=ecs[:, c, off:off + ln],
                    start=True, stop=True,
                    tile_position=(0, 32 * j),
                )
            nc.vector.tensor_copy(
                as_sb[:, half * ln:(half + 1) * ln], as_ps[:, 0:ln])

        def dma_out_block(prev, as_sb):
            e_sb, pr, t, off, ln = prev
            hbm = out[pr].rearrange("(h p) n -> p h n", p=128)[
                :, :, t * TCH + off:t * TCH + off + ln
            ]
            sb = as_sb[:, :2 * ln].rearrange("p (h n) -> p h n", h=2)
            nc.sync.dma_start(hbm, sb)

        def body():
            # dummy activation first: pulls the exp table load under the DMAs
            warm = warm_pool.tile([128, 4], F32, tag="warm")
            nc.vector.memset(warm[:, 0:2], 0.0)
            nc.scalar.activation(
                warm[:, 2:4], warm[:, 0:2], mybir.ActivationFunctionType.Exp,
            )

            inp_sb = const_pool.tile([128, INW], in_dt, tag="inp_sb")
            # pair-0 operands first, then W, then pair-1
            nc.sync.dma_start(inp_sb[:, LT_OFF:LT_OFF + L],
                              inp[:, LT_OFF:LT_OFF + L])
            nc.sync.dma_start(inp_sb[:, XT_OFF:XT_OFF + TCH],
                              inp[:, XT_OFF:XT_OFF + TCH])
            nc.sync.dma_start(inp_sb[:, XT_OFF + TCH:XT_OFF + SEQ2],
                              inp[:, XT_OFF + TCH:XT_OFF + SEQ2])
            nc.sync.dma_start(inp_sb[:, W_OFF:INW], inp[:, W_OFF:INW])
            nc.sync.dma_start(inp_sb[:, LT_OFF + L:LT_OFF + 2 * L],
                              inp[:, LT_OFF + L:LT_OFF + 2 * L])
            nc.sync.dma_start(inp_sb[:, XT_OFF + SEQ2:XT_OFF + 2 * SEQ2],
                              inp[:, XT_OFF + SEQ2:XT_OFF + 2 * SEQ2])
            w_sb = inp_sb[:, W_OFF:INW]

            if _parts == "dma":
                return
            do_exp = _parts in ("exp", "mm2", "full")
            do_mm2 = _parts in ("mm2", "full")
            do_out = _parts == "full"

            prev = None          # (e_sb, pr, t, off, ln)
            prev_ps = None       # (as_ps, as_sb) of prev

            for pr, t, off, ln in blocks:
                xt = inp_sb[:, XT_OFF + pr * SEQ2:XT_OFF + (pr + 1) * SEQ2]
                lt = inp_sb[:, LT_OFF + pr * L:LT_OFF + (pr + 1) * L]
                tok = slice(t * TCH + off, t * TCH + off + ln)
                e_sb = esb_pool.tile([128, 8 * TCH], in_dt, tag="esb")
                ecs = e_sb[:].rearrange("p (c n) -> p c n", c=8)
                groups = [(0, 1, 2), (3, 4, 5), (6, 7)]
                for gidx, grp in enumerate(groups):
                    e_ps = eps_pool.tile([128, TCH * 3], F32, tag="eps")
                    # chunk outputs at bank-aligned offsets (gi * TCH even for
                    # short sub-blocks): concurrent row-group matmuls must not
                    # write the same PSUM bank
                    epv = e_ps[:].rearrange("p (g n) -> p g n", g=3)
                    for gi, c in enumerate(grp):
                        half = c % 2
                        rows = slice(64 * half, 64 * (half + 1))
                        nc.tensor.matmul(
                            e_ps[:, gi * TCH:gi * TCH + ln],
                            lhsT=lt[rows, 128 * c:128 * (c + 1)],
                            rhs=xt[rows, tok],
                            start=True, stop=True,
                            tile_position=(64 * half, 0),
                        )
                    if not do_exp:
                        continue
                    e_out = ecs[:, grp[0]:grp[-1] + 1, off:off + ln]
                    if gidx == 2 and DVE_EXP:
                        nc.vector.tensor_scalar(
                            e_out.bitcast(I16),
                            epv[:, 0:2, 0:ln],
                            SCHRAU_A, SCHRAU_B,
                            mybir.AluOpType.mult, mybir.AluOpType.add,
                        )
                    else:
                        nc.scalar.activation(
                            e_out, epv[:, 0:len(grp), 0:ln],
                            mybir.ActivationFunctionType.Exp,
                        )
                    # previous block's MM2 halves slot between MM1 groups so
                    # the PE queue stays ahead of the exp engines
                    if do_mm2 and prev is not None and gidx in (1, 2):
                        if gidx == 1:
                            as_sb = assb_pool.tile([128, 2 * TCH], out_dt, tag="assb")
                            prev_ps = as_sb
                        issue_mm2_half(w_sb, prev, gidx - 1, prev_ps)
                if do_mm2 and prev is not None:
                    if do_out:
                        dma_out_block(prev, prev_ps)
                prev = (e_sb, pr, t, off, ln)

            if do_mm2:
                # drain: last block's MM2 + copies + DMA
                as_sb = assb_pool.tile([128, 2 * TCH], out_dt, tag="assb")
                for half in range(2):
                    issue_mm2_half(w_sb, prev, half, as_sb)
                if do_out:
                    dma_out_block(prev, as_sb)

        if loop_n > 1:
            with tc.For_i(0, loop_n):
                body()
        else:
            body()
    nc.compile()
    return nc


def _get_nc(loop_n=1, _parts="full"):
    key = (MM_MODE, OUT_BF16, DVE_EXP, loop_n, _parts)
    if key not in _nc_cache:
        _nc_cache[key] = _build_nc(loop_n, _parts)
    return _nc_cache[key]


_runner_cache = {}


def _get_runner(loop_n=1, _parts="full"):
    """Build (once) a jitted shard_map callable over the 8 cores, mirroring
    bass2jax.run_bass_via_pjrt but cached so repeat calls don't re-trace."""
    key = (MM_MODE, OUT_BF16, DVE_EXP, loop_n, _parts)
    if key in _runner_cache:
        return _runner_cache[key]
    import jax
    from jax.sharding import Mesh, PartitionSpec
    try:
        from jax.experimental.shard_map import shard_map
    except ImportError:
        from jax.shard_map import shard_map  # newer jax
    from concourse import bass2jax as b2j

    b2j.install_neuronx_cc_hook()
    nc = _get_nc(loop_n, _parts)

    partition_name = (
        nc.partition_id_tensor.name if nc.partition_id_tensor else None
    )
    in_names, out_names, out_avals, zero_shapes = [], [], [], []
    for alloc in nc.m.functions[0].allocations:
        if not isinstance(alloc, mybir.MemoryLocationSet):
            continue
        name = alloc.memorylocations[0].name
        if alloc.kind == "ExternalInput":
            if name != partition_name:
                in_names.append(name)
        elif alloc.kind == "ExternalOutput":
            out_names.append(name)
            shape = tuple(alloc.tensor_shape)
            dtype = mybir.dt.np(alloc.dtype)
            out_avals.append(jax.core.ShapedArray(shape, dtype))
            zero_shapes.append((shape, dtype))
    n_params = len(in_names)
    n_outs = len(out_avals)
    all_names = list(in_names) + list(out_names)
    if partition_name is not None:
        all_names.append(partition_name)
    donate = tuple(range(n_params, n_params + n_outs))

    def _body(*args):
        operands = list(args)
        if partition_name is not None:
            operands.append(b2j.partition_id_tensor())
        outs = b2j._bass_exec_p.bind(
            *operands,
            out_avals=tuple(out_avals),
            in_names=tuple(all_names),
            out_names=tuple(out_names),
            lowering_input_output_aliases=(),
            sim_require_finite=True,
            sim_require_nnan=True,
            nc=nc,
        )
        return tuple(outs)

    devices = jax.devices()[:NCORES]
    mesh = Mesh(np.asarray(devices), ("core",))
    in_specs = (PartitionSpec("core"),) * (n_params + n_outs)
    out_specs = (PartitionSpec("core"),) * n_outs
    sharded = jax.jit(
        shard_map(_body, mesh=mesh, in_specs=in_specs,
                  out_specs=out_specs, check_rep=False),
        donate_argnums=donate,
        keep_unused=True,
    )
    runner = {
        "jit": sharded, "in_names": in_names, "out_names": out_names,
        "out_avals": out_avals, "zero_shapes": zero_shapes, "mesh": mesh,
    }
    _runner_cache[key] = runner
    return runner


def _run_cores(in_maps):
    runner = _get_runner()
    concat_in = [
        np.concatenate([in_maps[c][name] for c in range(NCORES)], axis=0)
        for name in runner["in_names"]
    ]
    concat_zeros = [
        np.zeros((NCORES * s[0], *s[1:]), d) for (s, d) in runner["zero_shapes"]
    ]
    out_arrs = runner["jit"](*concat_in, *concat_zeros)
    results = []
    for c in range(NCORES):
        results.append({
            name: np.asarray(out_arrs[i]).reshape(
                NCORES, *runner["out_avals"][i].shape)[c]
            for i, name in enumerate(runner["out_names"])
        })
    return results


def _bench_setup(in_maps, loop_n, _parts="full"):
    import jax
    from jax.sharding import NamedSharding, PartitionSpec
    runner = _get_runner(loop_n, _parts)
    shard = NamedSharding(runner["mesh"], PartitionSpec("core"))
    concat_in = [
        np.concatenate([in_maps[c][name] for c in range(NCORES)], axis=0)
        for name in runner["in_names"]
    ]
    dev_in = [jax.device_put(a, shard) for a in concat_in]

    def zeros_dev():
        return [
            jax.device_put(np.zeros((NCORES * s[0], *s[1:]), d), shard)
            for (s, d) in runner["zero_shapes"]
        ]

    return runner["jit"], dev_in, zeros_dev


def _bench_round(fn, dev_in, zeros_dev, iters):
    import time as _time
    import jax
    zsets = [zeros_dev() for _ in range(iters)]
    jax.block_until_ready(zsets)
    t0 = _time.perf_counter()
    outs = [fn(*dev_in, *z) for z in zsets]
    jax.block_until_ready(outs)
    t1 = _time.perf_counter()
    return (t1 - t0) / iters


def benchmark_device(in_maps, r_small=128, r_big=1024, iters=4, rounds=4,
                     _parts="full"):
    """Per-iteration device exec time via the two-point slope of in-NEFF
    hardware loops: (T(r_big) - T(r_small)) / (r_big - r_small).  Per-call
    dispatch overhead (axon RTT, jax dispatch) cancels in the difference.
    The two loop sizes are measured in interleaved rounds so slow drift in
    dispatch overhead hits both equally; the median round slope is used."""
    fn_s, in_s, z_s = _bench_setup(in_maps, r_small, _parts)
    fn_b, in_b, z_b = _bench_setup(in_maps, r_big, _parts)
    # warm both
    _bench_round(fn_s, in_s, z_s, 1)
    _bench_round(fn_b, in_b, z_b, 1)
    slopes, t_smalls, t_bigs = [], [], []
    for _ in range(rounds):
        t_s = _bench_round(fn_s, in_s, z_s, iters)
        t_b = _bench_round(fn_b, in_b, z_b, iters)
        t_smalls.append(t_s)
        t_bigs.append(t_b)
        slopes.append((t_b - t_s) / (r_big - r_small))
    slopes.sort()
    t_iter = slopes[len(slopes) // 2]
    return t_iter, min(t_smalls), min(t_bigs)


def benchmark(in_maps, iters=16):
    """Legacy single-shot estimate: pipelined per-call slope (includes host
    dispatch; see benchmark_device for the amortized device-only number)."""
    import time as _time
    import jax
    from jax.sharding import NamedSharding, PartitionSpec
    runner = _get_runner()
    mesh = runner["mesh"]
    shard = NamedSharding(mesh, PartitionSpec("core"))
    concat_in = [
        np.concatenate([in_maps[c][name] for c in range(NCORES)], axis=0)
        for name in runner["in_names"]
    ]
    dev_in = [jax.device_put(a, shard) for a in concat_in]
    fn = runner["jit"]

    def zeros_dev():
        return [
            jax.device_put(np.zeros((NCORES * s[0], *s[1:]), d), shard)
            for (s, d) in runner["zero_shapes"]
        ]

    out = fn(*dev_in, *zeros_dev())
    jax.block_until_ready(out)
    zsets = [zeros_dev() for _ in range(iters)]
    jax.block_until_ready(zsets)
    outs = []
    t0 = _time.perf_counter()
    for z in zsets:
        outs.append(fn(*dev_in, *z))
    jax.block_until_ready(outs)
    t1 = _time.perf_counter()
    per_call = (t1 - t0) / iters
    z = zeros_dev()
    jax.block_until_ready(z)
    t2 = _time.perf_counter()
    out = fn(*dev_in, *z)
    jax.block_until_ready(out)
    t3 = _time.perf_counter()
    return per_call, (t3 - t2)


def _np_in_dt():
    if MM_MODE == "bf16":
        import ml_dtypes
        return ml_dtypes.bfloat16
    return np.float32


def _prep_core_inputs(Q, K, sketching_matrix, random_sign):
    """Host-side shard prep: per core one packed [128, INW] array."""
    X = np.concatenate([np.asarray(Q, np.float32),
                        np.asarray(K, np.float32)], axis=2)  # [B,H,4096,64]
    sk = np.asarray(sketching_matrix).astype(np.int64)       # [B, M, D]
    sign = np.asarray(random_sign, dtype=np.float32)         # [M, D]

    # sign-delta weight matrix W[32m+dl, 32c+dl] = sign[m, 32c+dl]
    W = np.zeros((128, D), dtype=np.float32)
    for m in range(M):
        for c in range(D // 32):
            dl = np.arange(32)
            W[32 * m + dl, 32 * c + dl] = sign[m, 32 * c + dl]

    np_dt = _np_in_dt()
    in_maps = []
    for core in range(NCORES):
        packed = np.empty((128, INW), dtype=np.float32)
        for pr in range(PAIRS):
            pair = core * PAIRS + pr
            b, h = divmod(pair, H)
            Xp = X[b, h]                            # [4096, 64]
            xts = slice(XT_OFF + pr * SEQ2, XT_OFF + (pr + 1) * SEQ2)
            packed[0:64, xts] = Xp.T
            packed[64:128, xts] = Xp.T              # dup for PE row group 1
            lm = Xp[sk[b]]                          # [M, D, 64]
            # landmark order l' = 128c + 32m + dl where d = 32c + dl
            lmp = lm.reshape(M, D // 32, 32, P).transpose(1, 0, 2, 3)
            lmp = lmp.reshape(L, P)                 # [(c, m, dl), 64]
            lts = slice(LT_OFF + pr * L, LT_OFF + (pr + 1) * L)
            packed[0:64, lts] = lmp.T
            packed[64:128, lts] = lmp.T
        packed[:, W_OFF:INW] = W
        in_maps.append({"inp": packed.astype(np_dt)})
    return in_maps


def kernel(Q, K, sketching_matrix, random_sign):
    in_maps = _prep_core_inputs(Q, K, sketching_matrix, random_sign)
    results = _run_cores(in_maps)
    # unshard: device out [PAIRS, 256, 4096] (d-major) -> [B, H, 4096, 256]
    AS = np.empty((B, H, SEQ2, D), dtype=np.float32)
    for core in range(NCORES):
        o = results[core]["out"]                # [PAIRS, 256, 4096]
        for pr in range(PAIRS):
            pair = core * PAIRS + pr
            b, h = divmod(pair, H)
            AS[b, h] = o[pr].T.astype(np.float32)
    return AS
